# revision 5
# baseline (speedup 1.0000x reference)
"""AbundanceWeightedPooling Trainium2 kernel (8-core SPMD, n_otus-sharded).

Decomposition (per core, shard of N=1024 OTUs):
  device: tanh gate (4 ACT ops), logits = tanh * scores_bcast (DVE),
          e = exp(logits) (ACT), PE transposes of e, masked rounding copies,
          G_h = sum_n e_h[n,b] * [seq | 1][n,:] via f32r matmuls (PSUM accum).
          Outputs: e (bf16, for avg_attn) and G partials [64, 4*257].
  host:   scores = seq @ score_W.T + score_b (tiny), input tiling,
          sum of G partials over cores, value/out projections, gelu+LN on
          [64,256], avg_attn assembly. No cross-core collectives.
"""
import sys
import os

sys.path.insert(0, "/opt/trn_rl_repo")

import numpy as np

N_CORES = 8
N_OTUS, B, SEQ_DIM, EMBED_DIM, N_HEADS = 8192, 64, 256, 256, 4
HEAD_DIM = EMBED_DIM // N_HEADS
LN_EPS = 1e-5
NSH = N_OTUS // N_CORES        # 1024 OTUs per core
NHALF = NSH // 2               # 512
NCHUNK = NSH // 128            # 8 chunks of 128 rows
SEQ_AUG = SEQ_DIM + 2          # 258 (ones cols; fp32r matmul needs even dims)

_CACHE = {}


def _build(gate_w: np.ndarray, gate_b: np.ndarray):
    """Build the Bacc module. gate_w/gate_b are baked as ACT immediates."""
    import concourse.bass as bass
    import concourse.tile as tile
    from concourse.bacc import Bacc
    from concourse import mybir
    from concourse.masks import make_identity

    dt = mybir.dt
    AF = mybir.ActivationFunctionType

    nc = Bacc()
    # per-core inputs
    d_seq = nc.dram_tensor("seq_aug", [NSH, SEQ_AUG], dt.float32r, kind="ExternalInput")
    d_clr = nc.dram_tensor("clr_t", [128, NHALF], dt.float32, kind="ExternalInput")
    d_msk = nc.dram_tensor("mask_t", [128, NHALF], dt.float32, kind="ExternalInput")
    d_sco = nc.dram_tensor("scores_t", [128, N_HEADS * NHALF], dt.float32, kind="ExternalInput")
    # per-core outputs
    d_e = nc.dram_tensor("e_out", [128, N_HEADS * NHALF], dt.bfloat16, kind="ExternalOutput")
    d_g = nc.dram_tensor("g_out", [B, N_HEADS * SEQ_AUG], dt.float32, kind="ExternalOutput")

    with tile.TileContext(nc) as tc:
        with (
            tc.tile_pool(name="cst", bufs=1) as cst,
            tc.tile_pool(name="sb", bufs=1) as sb,
            tc.tile_pool(name="ps", bufs=2, space="PSUM") as ps,
            tc.tile_pool(name="psg", bufs=1, space="PSUM") as psg,
        ):
            ident = cst.tile([128, 128], dt.float32)
            make_identity(nc, ident)

            t_clr = sb.tile([128, NHALF], dt.float32)
            t_msk = sb.tile([128, NHALF], dt.float32)
            t_sco = sb.tile([128, N_HEADS * NHALF], dt.float32)
            t_seq = sb.tile([128, NCHUNK * SEQ_AUG], dt.float32r)
            nc.sync.dma_start(out=t_clr[:], in_=d_clr[:])
            nc.sync.dma_start(out=t_msk[:], in_=d_msk[:])
            nc.sync.dma_start(out=t_sco[:], in_=d_sco[:])
            # seq rows n = chunk*128 + p  ->  tile[p, chunk*257 + col]
            seq_src = bass.AP(
                tensor=d_seq,
                offset=0,
                ap=[[SEQ_AUG, 128], [128 * SEQ_AUG, NCHUNK], [1, SEQ_AUG]],
            )
            nc.sync.dma_start(out=t_seq[:], in_=seq_src)

            # tanh gate: one ACT op per head, shared clr input (scale/bias immediates)
            t_tanh = sb.tile([128, N_HEADS * NHALF], dt.float32)
            for h in range(N_HEADS):
                nc.scalar.activation(
                    out=t_tanh[:, h * NHALF:(h + 1) * NHALF],
                    in_=t_clr[:],
                    func=AF.Tanh,
                    bias=float(gate_b[h]),
                    scale=float(gate_w[h]),
                )

            # logits = tanh * scores (scores pre-broadcast over b on host)
            t_log = sb.tile([128, N_HEADS * NHALF], dt.float32)
            nc.vector.tensor_tensor(
                out=t_log[:], in0=t_tanh[:], in1=t_sco[:], op=mybir.AluOpType.mult
            )

            # e = exp(logits)  (unmasked; mask applied to transposed copies + on host)
            t_e = sb.tile([128, N_HEADS * NHALF], dt.float32)
            nc.scalar.activation(out=t_e[:], in_=t_log[:], func=AF.Exp)

            # e out (bf16 cast on SWDGE dma)
            nc.gpsimd.dma_start(out=d_e[:], in_=t_e[:])

            # notmask transposed: 4 PE transposes [128(cb),128(n'')] -> [n'', cb]
            p_nm = ps.tile([128, NHALF], dt.float32, tag="pnm")
            for blk in range(4):
                nc.tensor.transpose(
                    p_nm[:, blk * 128:(blk + 1) * 128],
                    t_msk[:, blk * 128:(blk + 1) * 128],
                    ident[:],
                )
            t_nmT = sb.tile([128, NHALF], dt.float32)
            nc.vector.tensor_copy(out=t_nmT[:], in_=p_nm[:])

            # per head: transpose e, mask+round to f32r, matmul-accumulate G
            for h in range(N_HEADS):
                p_eT = ps.tile([128, NHALF], dt.float32, tag="peT")
                for blk in range(4):
                    nc.tensor.transpose(
                        p_eT[:, blk * 128:(blk + 1) * 128],
                        t_e[:, h * NHALF + blk * 128: h * NHALF + (blk + 1) * 128],
                        ident[:],
                    )
                t_eT = sb.tile([128, NHALF], dt.float32r, tag="teT")
                nc.vector.tensor_tensor(
                    out=t_eT[:], in0=p_eT[:], in1=t_nmT[:], op=mybir.AluOpType.mult
                )
                p_g = psg.tile([B, SEQ_AUG], dt.float32, tag=f"pg{h}")
                for blk in range(4):
                    for c in range(2):
                        k = c * 4 + blk  # global chunk: n = c*512 + blk*128 + n''
                        nc.tensor.matmul(
                            p_g[:],
                            t_eT[:, blk * 128 + c * 64: blk * 128 + (c + 1) * 64],
                            t_seq[:, k * SEQ_AUG:(k + 1) * SEQ_AUG],
                            start=(blk == 0 and c == 0),
                            stop=(blk == 3 and c == 1),
                        )
                t_g = sb.tile([B, SEQ_AUG], dt.float32, tag=f"tg{h}")
                nc.vector.tensor_copy(out=t_g[:], in_=p_g[:])
                nc.sync.dma_start(
                    out=d_g[:, h * SEQ_AUG:(h + 1) * SEQ_AUG], in_=t_g[:]
                )

    nc.finalize()
    return nc


def _get_nc(gate_w, gate_b):
    key = (tuple(np.asarray(gate_w).ravel().tolist()), tuple(np.asarray(gate_b).ravel().tolist()))
    if key not in _CACHE:
        _CACHE[key] = _build(np.asarray(gate_w, np.float32).ravel(), np.asarray(gate_b, np.float32).ravel())
    return _CACHE[key]


def kernel(sequence_embeddings, clr_abundances, padding_mask,
           score_W, score_b, gate_W, gate_b, value_W, value_b,
           out_W, out_b, ln_gamma, ln_beta):
    from concourse.bass_utils import run_bass_kernel_spmd

    seq = np.asarray(sequence_embeddings, np.float32)
    clr = np.asarray(clr_abundances, np.float32)
    mask = np.asarray(padding_mask)
    score_W = np.asarray(score_W, np.float32)
    score_b = np.asarray(score_b, np.float32)
    gate_w = np.asarray(gate_W, np.float32)[:, 0]
    gate_bv = np.asarray(gate_b, np.float32)
    value_W_ = np.asarray(value_W, np.float32)
    value_b_ = np.asarray(value_b, np.float32)
    out_W_ = np.asarray(out_W, np.float32)
    out_b_ = np.asarray(out_b, np.float32)
    gam = np.asarray(ln_gamma, np.float32)
    bet = np.asarray(ln_beta, np.float32)

    nc = _get_nc(gate_w, gate_bv)

    # ---- host prep ----
    scores = seq @ score_W.T + score_b                       # [N, H]
    seq_aug = np.empty((N_OTUS, SEQ_AUG), np.float32)
    seq_aug[:, :SEQ_DIM] = seq
    seq_aug[:, SEQ_DIM:] = 1.0

    # clr/mask tiles: [core, (c,b), n'] with n = core*1024 + c*512 + n'
    clr_t = np.ascontiguousarray(
        clr.reshape(B, N_CORES, 2, NHALF).transpose(1, 2, 0, 3)
    ).reshape(N_CORES, 128, NHALF)
    nmask_t = np.ascontiguousarray(
        (~mask).astype(np.float32).reshape(B, N_CORES, 2, NHALF).transpose(1, 2, 0, 3)
    ).reshape(N_CORES, 128, NHALF)
    # scores tiles: [core, (c,b), (h, n')], broadcast over b
    sco_r = scores.reshape(N_CORES, 2, NHALF, N_HEADS).transpose(0, 1, 3, 2)  # [core,c,h,n']
    sco_t = np.ascontiguousarray(
        np.broadcast_to(sco_r[:, :, None, :, :], (N_CORES, 2, B, N_HEADS, NHALF))
    ).reshape(N_CORES, 128, N_HEADS * NHALF)

    in_maps = [
        {
            "seq_aug": seq_aug[c * NSH:(c + 1) * NSH],
            "clr_t": clr_t[c],
            "mask_t": nmask_t[c],
            "scores_t": sco_t[c],
        }
        for c in range(N_CORES)
    ]
    res = run_bass_kernel_spmd(nc, in_maps, core_ids=list(range(N_CORES)))

    # ---- host finalize ----
    g = np.zeros((B, N_HEADS, SEQ_AUG), np.float32)
    for c in range(N_CORES):
        g += res.results[c]["g_out"].reshape(B, N_HEADS, SEQ_AUG)
    D = g[:, :, SEQ_DIM]                                    # [B, H]
    # weighted[b,h,d] = sum_k G[b,h,k] * value_W[h*64+d,k] + D*value_b
    vW = value_W_.reshape(N_HEADS, HEAD_DIM, SEQ_DIM)
    weighted = np.einsum("bhk,hdk->bhd", g[:, :, :SEQ_DIM], vW, optimize=True)
    weighted += D[:, :, None] * value_b_.reshape(N_HEADS, HEAD_DIM)
    pooled = (weighted / D[:, :, None]).reshape(B, EMBED_DIM)

    hlin = pooled @ out_W_.T + out_b_
    # exact gelu via erf
    from math import sqrt
    try:
        from scipy.special import erf as _erf
        erf_v = _erf(hlin / sqrt(2.0))
    except Exception:
        import math
        erf_v = np.vectorize(math.erf)(hlin / sqrt(2.0))
    gelu = 0.5 * hlin * (1.0 + erf_v)
    mu = gelu.mean(-1, keepdims=True)
    var = gelu.var(-1, keepdims=True)
    output = ((gelu - mu) / np.sqrt(var + LN_EPS) * gam + bet).astype(np.float32)

    # avg_attn from e outputs
    e_all = np.stack([res.results[c]["e_out"] for c in range(N_CORES)])  # [8,128,2048] bf16->f32
    e_all = e_all.astype(np.float32).reshape(N_CORES, 2, B, N_HEADS, NHALF)
    e_bnh = e_all.transpose(2, 0, 1, 4, 3).reshape(B, N_OTUS, N_HEADS)
    e_bnh *= (~mask)[:, :, None]
    avg_attn = (e_bnh / D[:, None, :]).mean(-1).astype(np.float32)

    return output, avg_attn
